# revision 7
# baseline (speedup 1.0000x reference)
"""CAMSA multi-mask attention kernel for one TRN2 chip (8 NeuronCores).

Problem: B=4, S=2048, D=1024, M=4 stride masks.
  Q = x@Wq ; K = x@Wk ; V = x@Wv      (biases zero-fill)
  scores = Q K^T / sqrt(D)
  weights_m = softmax(where(mask_m==0, -1e9, scores))
  out = (mean_m weights_m) @ V @ Wo

Algebra (v2): Q/K/V are never materialized.
  G = Wq Wk^T ; H = Wv Wo            (weight folding, [D,D] each)
  scores = (x G) x^T                 -> T = xq@G, scores = T x^T
  out    = (Wsum x) H                -> U = Wsum@x, out = U H
P = exp(scores/sqrt(D))/M (the 1/M folds into wvt); per-mask
  den_m[q] = sum_k mask_m[q,k] P[q,k];  inv_m = 1/den_m
  Wsum = sum_m inv_m * (mask_m*P);  out = Wsum @ x @ H

Sharding: core c = (batch b=c//2, query-half h=c%2): 1024 query rows,
full 2048 keys; no collectives.

Schedule (v3): the tensor engine is the critical engine (228us busy of
284us span in v2); v3 makes its stream gap-free:
  tensor order: G -> TT -> scores(t=0..7) -> H -> f0 -> g0..3 -> f1 -> g4..7
  - H (Wv@Wo) moved AFTER the scores loop (emitted in chunks inside
    loop iters t=4..7 so its ACT drains interleave with the scales):
    the DVE softmax pipeline starts ~27us earlier and hides fully
    under tensor work (v2 had an 18us tensor gap waiting on it).
  - masks host-cast to BF16 (not u8): every non-scalar DVE operand is
    2-byte so the mask*P STTs hit the 2X_1PORT mode (~1.2us vs 2.28us
    per [128,2048] plane; u8 operands force 1x). +8.4MB DMA, cheap.
  - head: first G chain is gated on 1.25MB (one 128-col slice of wqt on
    the gpsimd ring + the jb=0 half of wkt on the sync ring) instead of
    4MB on one ring (v2 started at 17.8us).
  - xk (F-phase layout of x) shares its 32KB SBUF slot with xTs
    (scores-phase layout); the refill DMA is triggered from the gpsimd
    ring after the last mask trigger, where its WAR-wait on the xTs
    readers blocks nothing, so it fires at scores-end and lands before
    f0. This frees the room to keep wv/wo resident for the deferred H.
  - exp runs 4 tiles ahead so the ACT queue (exp+scales) never
    back-pressures the scores stream through the 8-slot PSUM ring.
  - recip(t) is emitted before adds(t-1) on DVE so scale(t) (ACT) is
    never serialized behind the adds.
"""

import numpy as np

B, S, D, M = 4, 2048, 1024, 4
SQ = S // 2          # query rows per core
PART = 128
N_CORES = 8

_CACHE = {}


def build(nc_factory=None, use_deps=True):
    from concourse import bass, mybir, bacc, tile
    from concourse.tile import add_dep_helper

    fp32 = mybir.dt.float32
    bf16 = mybir.dt.bfloat16
    AF = mybir.ActivationFunctionType
    ALU = mybir.AluOpType

    P = PART
    DCH = D // P         # d-chunks (8)
    KCH = S // P         # key-row chunks (16)
    QTILES = SQ // P     # q-tiles per core (8)
    NB = 512

    if nc_factory is None:
        nc = bacc.Bacc("TRN2", target_bir_lowering=False, debug=False,
                       num_devices=N_CORES)
    else:
        nc = nc_factory()

    xT_d = nc.dram_tensor("xT", [D, S], bf16, kind="ExternalInput")
    xTq_d = nc.dram_tensor("xTq", [D, SQ], bf16, kind="ExternalInput")
    xk_d = nc.dram_tensor("xk", [S, D], bf16, kind="ExternalInput")
    mk_d = nc.dram_tensor("mk", [M, SQ, S], bf16, kind="ExternalInput")
    wqt_d = nc.dram_tensor("wqt", [D, D], bf16, kind="ExternalInput")
    wkt_d = nc.dram_tensor("wkt", [D, D], bf16, kind="ExternalInput")
    wvt_d = nc.dram_tensor("wvt", [D, D], bf16, kind="ExternalInput")
    wo_d = nc.dram_tensor("wo", [D, D], bf16, kind="ExternalInput")
    out_d = nc.dram_tensor("out", [SQ, D], fp32, kind="ExternalOutput")

    ncopy = [0]

    with tile.TileContext(nc) as tc:
        with tc.tile_pool(name="persist", bufs=1) as pp, \
             tc.tile_pool(name="psum", bufs=8, space="PSUM") as psp:

            # xTs and xks share one 32KB slot (tag xmem): xTs is dead after
            # the last scores matmul; the xk DMA (triggered from the tensor
            # queue) then refills the slot for phase F.
            xTs = pp.tile([P, DCH * S], bf16, tag="xmem")  # [p,c*S+k]=xT[c*128+p,k]
            TT = pp.tile([P, DCH * SQ], bf16)    # [p, c*SQ+q] = T[q, c*128+p]
            Hs = pp.tile([P, DCH * D], bf16)     # [p, c*D+o]  = H[c*128+p, o]

            def drain(dst, ps, eng=None):
                # PSUM -> SBUF copies (gpsimd has no PSUM port: DVE/ACT only)
                if eng is None:
                    ncopy[0] += 1
                    eng = nc.vector if ncopy[0] % 2 else nc.scalar
                if eng is nc.scalar:
                    nc.scalar.activation(dst, ps, AF.Copy, scale=1.0)
                else:
                    eng.tensor_copy(dst, ps)

            # WT written during phase C (tail transposes), read in F. Opened
            # before wvwo so pool releases stay LIFO (wvwo closes first).
            wt_ctx = tc.tile_pool(name="wtp", bufs=1)
            wtp = wt_ctx.__enter__()
            WT = wtp.tile([P, KCH * SQ], bf16, name="WT")  # [p,i*SQ+q]=Wsum[q,i*128+p]

            # wv/wo live from their (early) DMA until H is emitted inside
            # the phase-C loop.
            wvwo_ctx = tc.tile_pool(name="wvwo", bufs=1)
            wvp = wvwo_ctx.__enter__()
            wv2 = wvp.tile([P, DCH * D], bf16, name="wv2")
            wo2 = wvp.tile([P, DCH * D], bf16, name="wo2")

            # ---- phase B: G = Wq Wk^T ; TT = (xq G)^T ------------------
            with tc.tile_pool(name="stage", bufs=1) as sw:
                wa = sw.tile([P, DCH * D], bf16, name="wa")
                wb = sw.tile([P, DCH * D], bf16, name="wb")
                Gs = sw.tile([P, DCH * D], bf16, name="Gs")
                xTq = sw.tile([P, DCH * SQ], bf16, name="xTq")

                # gpsimd ring: wa in 128-col slices (first G chain needs
                # only slice 0: 256KB), then xTq, then (in-loop) the masks.
                gp_dmas = []
                for s8 in range(DCH):
                    lo, hi = s8 * P, (s8 + 1) * P
                    gp_dmas.append(nc.gpsimd.dma_start(
                        wa[:].rearrange("p (c d) -> p c d", c=DCH)[:, :, lo:hi],
                        wqt_d.ap()[:, lo:hi].rearrange("(c p) d -> p c d", p=P)))
                gp_dmas.append(nc.gpsimd.dma_start(
                    xTq[:].rearrange("p (c r) -> p c r", c=DCH),
                    xTq_d.ap().rearrange("(c p) r -> p c r", p=P)))

                # sync ring: wb jb-halves (first G chain needs half 0),
                # then xTs, then wv/wo for the deferred H.
                sy_dmas = []
                for half in range(2):
                    lo, hi = half * (D // 2), (half + 1) * (D // 2)
                    sy_dmas.append(nc.sync.dma_start(
                        wb[:].rearrange("p (c d) -> p c d", c=DCH)[:, :, lo:hi],
                        wkt_d.ap()[:, lo:hi].rearrange("(c p) d -> p c d", p=P)))
                sy_dmas.append(nc.sync.dma_start(
                    xTs[:].rearrange("p (c r) -> p c r", c=DCH),
                    xT_d.ap().rearrange("(c p) r -> p c r", p=P)))
                sy_dmas.append(nc.sync.dma_start(
                    wv2[:].rearrange("p (c d) -> p c d", c=DCH),
                    wvt_d.ap().rearrange("(c p) d -> p c d", p=P)))
                sy_dmas.append(nc.sync.dma_start(
                    wo2[:].rearrange("p (c d) -> p c d", c=DCH),
                    wo_d.ap().rearrange("(c p) d -> p c d", p=P)))
                if use_deps:
                    for ring in (gp_dmas, sy_dmas):
                        for a, b in zip(ring[1:], ring[:-1]):
                            add_dep_helper(a.ins, b.ins, sync=False,
                                           reason="dma order")

                # G[i,j] = sum_d Wq[i,d] Wk[j,d]: lhsT=WqT slice, rhs=WkT
                for jb in range(D // NB):
                    for ic in range(DCH):
                        ps = psp.tile([P, NB], fp32, tag="ps", name="ps")
                        for c in range(DCH):
                            nc.tensor.matmul(
                                ps[:],
                                wa[:, c * D + ic * P: c * D + (ic + 1) * P],
                                wb[:, c * D + jb * NB: c * D + (jb + 1) * NB],
                                start=(c == 0), stop=(c == DCH - 1))
                        drain(Gs[:, ic * D + jb * NB: ic * D + (jb + 1) * NB], ps[:])
                # TT[j,q] = sum_i G[i,j] xq[q,i]: lhsT=G chunk, rhs=xTq
                for jc in range(DCH):
                    for qb in range(SQ // NB):
                        ps = psp.tile([P, NB], fp32, tag="ps", name="ps")
                        for ic in range(DCH):
                            nc.tensor.matmul(
                                ps[:],
                                Gs[:, ic * D + jc * P: ic * D + (jc + 1) * P],
                                xTq[:, ic * SQ + qb * NB: ic * SQ + (qb + 1) * NB],
                                start=(ic == 0), stop=(ic == DCH - 1))
                        drain(TT[:, jc * SQ + qb * NB: jc * SQ + (qb + 1) * NB], ps[:])

            # ---- phases C/E (+ H interleaved) ---------------------------
            wk_ctx = tc.tile_pool(name="work", bufs=2)
            wkp = wk_ctx.__enter__()

            def mt_load(t):
                # gpsimd queue: gp does no elementwise work in phase C, so
                # its queue is free for the mask DMA triggers
                mt = wkp.tile([P, M * S], bf16, tag="mt", name=f"mt{t}", bufs=2)
                nc.gpsimd.dma_start(
                    mt[:].rearrange("p (m k) -> p m k", m=M),
                    mk_d.ap()[:, t * P:(t + 1) * P, :].transpose([1, 0, 2]))
                return mt

            inv_scale = 1.0 / float(np.sqrt(np.float32(D)))
            mts = {0: mt_load(0), 1: mt_load(1)}
            pts = {}

            def sc_exp(t):
                """scores tile -> exp -> Pt (emitted 4 tiles ahead so the
                ACT queue's exp stream never gates the scores stream)."""
                Pt = wkp.tile([P, S], bf16, tag="Pt", name="Pt", bufs=3)
                for kb in range(S // NB):
                    ps = psp.tile([P, NB], fp32, tag="ps", name="ps")
                    for c in range(DCH):
                        nc.tensor.matmul(
                            ps[:],
                            TT[:, c * SQ + t * P: c * SQ + (t + 1) * P],
                            xTs[:, c * S + kb * NB: c * S + (kb + 1) * NB],
                            start=(c == 0), stop=(c == DCH - 1))
                    nc.scalar.activation(
                        Pt[:, kb * NB:(kb + 1) * NB], ps[:],
                        AF.Exp, scale=inv_scale)
                pts[t] = Pt

            def c_prod(t):
                """tile t products: fused mask*P + row-sums, all on DVE.
                GpSimd does NO elementwise work in phase C: it shares SBUF
                ports with DVE, so concurrent gp ops halve DVE throughput.
                (The softmax-mean's 1/M is folded into wvt on the host.)"""
                if t + 2 < QTILES:
                    mts[t + 2] = mt_load(t + 2)
                mt = mts.pop(t)
                Pt = pts.pop(t)

                den = wkp.tile([P, M], fp32, tag="den", name="den")
                Tm = [wkp.tile([P, S], bf16, tag=f"Tm{m}", name=f"Tm{m}",
                               bufs=2)
                      for m in range(M)]
                for m in range(M):
                    nc.vector.scalar_tensor_tensor(
                        out=Tm[m][:],
                        in0=mt[:, m * S:(m + 1) * S],
                        scalar=1.0, in1=Pt[:],
                        op0=ALU.mult, op1=ALU.mult,
                        accum_out=den[:, m:m + 1])
                inv = wkp.tile([P, M], fp32, tag="inv", name="inv")
                nc.vector.reciprocal(inv[:], den[:])
                return inv, Tm

            def c_scale(t, inv, Tm):
                """per-mask inv scaling on ACT (own SBUF ports)."""
                for m in range(M):
                    nc.scalar.activation(Tm[m][:], Tm[m][:], AF.Copy,
                                         scale=inv[:, m:m + 1])
                return Tm

            def c_tail(t, Tm):
                nc.vector.tensor_add(Tm[0][:], Tm[0][:], Tm[1][:])
                nc.vector.tensor_add(Tm[2][:], Tm[2][:], Tm[3][:])
                nc.vector.tensor_add(Tm[0][:], Tm[0][:], Tm[2][:])
                # transpose Wsum [128, S] -> WT column t via xbar DMA
                nc.sync.dma_start_transpose(
                    WT[:].rearrange("p (i q) -> p i q", i=KCH)
                    [:, :, t * P:(t + 1) * P],
                    Tm[0][:])

            def h_chunk(k):
                # 2 ic-blocks of H = Wv Wo per call; drains on ACT where
                # they interleave with the scales (DVE is softmax-busy).
                for ic in (2 * k, 2 * k + 1):
                    for ob in range(D // NB):
                        ps = psp.tile([P, NB], fp32, tag="ps", name="ps")
                        for c in range(DCH):
                            nc.tensor.matmul(
                                ps[:],
                                wv2[:, c * D + ic * P: c * D + (ic + 1) * P],
                                wo2[:, c * D + ob * NB: c * D + (ob + 1) * NB],
                                start=(c == 0), stop=(c == DCH - 1))
                        drain(Hs[:, ic * D + ob * NB: ic * D + (ob + 1) * NB],
                              ps[:], eng=nc.scalar)

            # software pipeline: products(t) | tail(t-1) | scale(t) | exp(t+4)
            # - per-queue order keeps every engine's next op data-ready
            xks = None
            for t in range(4):
                sc_exp(t)
            prev = None
            for t in range(QTILES):
                inv, Tm = c_prod(t)
                if prev is not None:
                    c_tail(t - 1, prev)
                prev = c_scale(t, inv, Tm)
                if t + 4 < QTILES:
                    sc_exp(t + 4)
                if t == 5:
                    # xk refill of the xmem slot, triggered from the GPSIMD
                    # queue after the last mask trigger (mt(7), emitted at
                    # t=5): its WAR-wait on the xTs readers blocks only the
                    # then-idle gp ring, so it fires at scores-end (~114us)
                    # and lands well before f0 needs it (~142us).
                    xks = pp.tile([P, KCH * D], bf16, tag="xmem", name="xks")
                    nc.gpsimd.dma_start(
                        xks[:].rearrange("p (i d) -> p i d", i=KCH),
                        xk_d.ap().rearrange("(i p) d -> p i d", p=P))
                if t >= 4:
                    h_chunk(t - 4)
            c_tail(QTILES - 1, prev)
            wk_ctx.__exit__(None, None, None)
            wvwo_ctx.__exit__(None, None, None)

            # ---- phases F/G ---------------------------------------------
            fg_ctx = tc.tile_pool(name="fg", bufs=1)
            fgp = fg_ctx.__enter__()
            OT = fgp.tile([P, DCH * NB], bf16, name="OT")  # [p,c*NB+qc]=U[qb*NB+qc,c*128+p]

            def f_block(qb):
                # OT[j, qc] = sum_k x[k, j*128+jj] Wsum[qb*NB+qc, k]
                for j in range(DCH):
                    ps = psp.tile([P, NB], fp32, tag="ps", name="ps")
                    for i in range(KCH):
                        nc.tensor.matmul(
                            ps[:],
                            xks[:, i * D + j * P: i * D + (j + 1) * P],
                            WT[:, i * SQ + qb * NB: i * SQ + (qb + 1) * NB],
                            start=(i == 0), stop=(i == KCH - 1))
                    drain(OT[:, j * NB:(j + 1) * NB], ps[:])

            def g_tile(t):
                ot = fgp.tile([P, D], fp32, tag="ot", name="ot", bufs=2)
                for ob in range(D // NB):
                    ps = psp.tile([P, NB], fp32, tag="ps", name="ps")
                    for c in range(DCH):
                        nc.tensor.matmul(
                            ps[:],
                            OT[:, c * NB + (t % 4) * P: c * NB + (t % 4 + 1) * P],
                            Hs[:, c * D + ob * NB: c * D + (ob + 1) * NB],
                            start=(c == 0), stop=(c == DCH - 1))
                    drain(ot[:, ob * NB:(ob + 1) * NB], ps[:])
                nc.sync.dma_start(out_d.ap()[t * P:(t + 1) * P, :], ot[:])

            for qb in range(SQ // NB):
                f_block(qb)
                for t in range(qb * (NB // P), (qb + 1) * (NB // P)):
                    g_tile(t)
            fg_ctx.__exit__(None, None, None)
            wt_ctx.__exit__(None, None, None)

    nc.compile()
    return nc


def _get_nc():
    if "nc" not in _CACHE:
        _CACHE["nc"] = build()
    return _CACHE["nc"]


def kernel(x, stride_masks, Wq, bq, Wk, bk, Wv, bv, Wo, bo):
    import ml_dtypes
    from concourse import bass_utils

    bf16 = ml_dtypes.bfloat16
    x = np.ascontiguousarray(np.asarray(x, dtype=np.float32))
    stride_masks = np.asarray(stride_masks, dtype=np.int32)
    Wq = np.asarray(Wq, dtype=np.float32)
    Wk = np.asarray(Wk, dtype=np.float32)
    Wv = np.asarray(Wv, dtype=np.float32)
    Wo = np.asarray(Wo, dtype=np.float32)
    bq = np.asarray(bq, dtype=np.float32)
    bk = np.asarray(bk, dtype=np.float32)
    bv = np.asarray(bv, dtype=np.float32)
    bo = np.asarray(bo, dtype=np.float32)

    nc = _get_nc()

    # Biases are spec'd zero-fill; the device kernel omits them. bv/bo fold
    # in exactly on the host (softmax rows sum to 1); bq/bk would need a
    # device path, so assert they are zero.
    assert not (np.any(bq) or np.any(bk)), "nonzero q/k bias unsupported"

    mk_bf = stride_masks.astype(bf16)   # 0/1 exact in bf16; enables DVE 2x
    mk_half = [np.ascontiguousarray(mk_bf[:, h * SQ:(h + 1) * SQ, :])
               for h in range(2)]
    wqt = Wq.T.astype(bf16)
    wkt = Wk.T.astype(bf16)
    wvt = (Wv.T / np.float32(M)).astype(bf16)   # folds the mask-mean 1/M
    wo16 = Wo.astype(bf16)
    xT_bf = [x[b].T.astype(bf16) for b in range(B)]
    xk_bf = [x[b].astype(bf16) for b in range(B)]

    in_maps = []
    for c in range(N_CORES):
        b, h = c // 2, c % 2
        in_maps.append({
            "xT": xT_bf[b],
            "xTq": np.ascontiguousarray(xT_bf[b][:, h * SQ:(h + 1) * SQ]),
            "xk": xk_bf[b], "mk": mk_half[h],
            "wqt": wqt, "wkt": wkt, "wvt": wvt, "wo": wo16,
        })

    res = bass_utils.run_bass_kernel_spmd(nc, in_maps, core_ids=list(range(N_CORES)))
    _CACHE["last_results"] = res

    out = np.empty((B, S, D), dtype=np.float32)
    for c in range(N_CORES):
        b, h = c // 2, c % 2
        out[b, h * SQ:(h + 1) * SQ, :] = res.results[c]["out"]

    if np.any(bv):
        out += (bv @ Wo)[None, None, :]
    if np.any(bo):
        out += bo[None, None, :]
    return out


# revision 13
# speedup vs baseline: 1.1167x; 1.1167x over previous
"""CAMSA multi-mask attention kernel for one TRN2 chip (8 NeuronCores).

Problem: B=4, S=2048, D=1024, M=4 stride masks.
  Q = x@Wq ; K = x@Wk ; V = x@Wv      (biases zero-fill)
  scores = Q K^T / sqrt(D)
  weights_m = softmax(where(mask_m==0, -1e9, scores))
  out = (mean_m weights_m) @ V @ Wo

Algebra (v2): Q/K/V are never materialized.
  G = Wq Wk^T ; H = Wv Wo            (weight folding, [D,D] each)
  scores = (x G) x^T                 -> T = xq@G, scores = T x^T
  out    = (Wsum x) H                -> U = Wsum@x, out = U H
P = exp(scores/sqrt(D))/M (the 1/M folds into wvt); per-mask
  den_m[q] = sum_k mask_m[q,k] P[q,k];  inv_m = 1/den_m
  Wsum = sum_m inv_m * (mask_m*P);  out = Wsum @ x @ H

Sharding: core c = (batch b=c//2, query-half h=c%2): 1024 query rows,
full 2048 keys; no collectives.

Schedule (v4): the tensor engine is the critical engine; the softmax
vector stream (DVE ~12.9us + ACT ~9us per 128-row q-tile; the mask STTs
run 1x regardless of dtype - 3-tensor ops have no DVE 2x path) must
start as early as possible and never gate the tensor stream:
  tensor order: G -> TTb0 -> scores(0-3) -> TTb1 -> scores(4-7)
                -> H (chunks) -> f0 -> g0-3 -> f1 -> g4-7
  - TT is computed in two 512-q blocks: scores(0) starts 14us earlier
    (right after TTb0), so the DVE softmax pipeline starts ~65us; the
    TTb1 bubble also lets the ACT exp stream catch up so the scores
    matmuls never stall on the 8-slot PSUM ring (v3 lost 14us there).
  - H = Wv Wo deferred: emitted in 4 chunks inside loop iters t=4..7
    (PSUM drains on ACT, interleaved between the scales).
  - first G chains run as two 256-wide PSUM half-chains so the first
    matmul is gated on a 512KB wkt slice + 256KB wqt slice across two
    DMA rings (v2/v3 waited ~17us on 2-4MB).
  - xk (F-phase layout of x) shares its 32KB SBUF slot with xTs
    (scores layout); the refill fires from the gp ring at ~130us
    (after mt(7)'s WAR releases) and lands before f0 (~155us).
  - f0's PSUM drains go to ACT, whose scale stream has ~30% idle gaps;
    f0's 8 chains exactly fill the 8 PSUM slots so the tensor engine
    never waits for a drain mid-block.
  - masks stay u8 (bf16 gives no STT speedup and costs 8.4MB DMA).
"""

import numpy as np

B, S, D, M = 4, 2048, 1024, 4
SQ = S // 2          # query rows per core
PART = 128
N_CORES = 8

_CACHE = {}


def build(nc_factory=None, use_deps=True):
    from concourse import bass, mybir, bacc, tile
    from concourse.tile import add_dep_helper

    fp32 = mybir.dt.float32
    bf16 = mybir.dt.bfloat16
    u8 = mybir.dt.uint8
    AF = mybir.ActivationFunctionType
    ALU = mybir.AluOpType

    P = PART
    DCH = D // P         # d-chunks (8)
    KCH = S // P         # key-row chunks (16)
    QTILES = SQ // P     # q-tiles per core (8)
    NB = 512

    if nc_factory is None:
        nc = bacc.Bacc("TRN2", target_bir_lowering=False, debug=False,
                       num_devices=N_CORES)
    else:
        nc = nc_factory()

    xT_d = nc.dram_tensor("xT", [D, S], bf16, kind="ExternalInput")
    xTq_d = nc.dram_tensor("xTq", [D, SQ], bf16, kind="ExternalInput")
    xk_d = nc.dram_tensor("xk", [S, D], bf16, kind="ExternalInput")
    mk_d = nc.dram_tensor("mk", [M, SQ, S], u8, kind="ExternalInput")
    wqt_d = nc.dram_tensor("wqt", [D, D], bf16, kind="ExternalInput")
    wkt_d = nc.dram_tensor("wkt", [D, D], bf16, kind="ExternalInput")
    wvt_d = nc.dram_tensor("wvt", [D, D], bf16, kind="ExternalInput")
    wo_d = nc.dram_tensor("wo", [D, D], bf16, kind="ExternalInput")
    out_d = nc.dram_tensor("out", [SQ, D], fp32, kind="ExternalOutput")

    ncopy = [0]

    with tile.TileContext(nc) as tc:
        with tc.tile_pool(name="persist", bufs=1) as pp, \
             tc.tile_pool(name="psum", bufs=8, space="PSUM") as psp:

            # xTs and xks share one 32KB slot (tag xmem): xTs is dead after
            # the last scores matmul; the xk DMA then refills the slot.
            xTs = pp.tile([P, DCH * S], bf16, tag="xmem")  # [p,c*S+k]=xT[c*128+p,k]
            TT = pp.tile([P, DCH * SQ], bf16)    # [p, c*SQ+q] = T[q, c*128+p]
            Hs = pp.tile([P, DCH * D], bf16)     # [p, c*D+o]  = H[c*128+p, o]

            def drain(dst, ps, eng=None):
                # PSUM -> SBUF copies (gpsimd has no PSUM port: DVE/ACT only)
                if eng is None:
                    ncopy[0] += 1
                    eng = nc.vector if ncopy[0] % 2 else nc.scalar
                if eng is nc.scalar:
                    nc.scalar.activation(dst, ps, AF.Copy, scale=1.0)
                else:
                    eng.tensor_copy(dst, ps)

            # WT written during phase C (tail transposes), read in F. Opened
            # before wvwo so pool releases stay LIFO (wvwo closes first).
            wt_ctx = tc.tile_pool(name="wtp", bufs=1)
            wtp = wt_ctx.__enter__()
            WT = wtp.tile([P, KCH * SQ], bf16, name="WT")  # [p,i*SQ+q]=Wsum[q,i*128+p]

            # ---- phase B: G = Wq Wk^T ; TT = (xq G)^T ------------------
            # stageG (Gs/xTq) lives through TTb1, then its tags are reused
            # for wv/wo (their DMAs WAR-wait on TTb1's last Gs/xTq reads,
            # landing ~115us, just before the H chunks at ~127us). stageW
            # (wa/wb) closes right after the G chains so the work pool can
            # reuse its bytes.
            stageg_ctx = tc.tile_pool(name="stageG", bufs=1)
            sg = stageg_ctx.__enter__()
            Gs = sg.tile([P, DCH * D], bf16, name="Gs", tag="Gs")
            xTq = sg.tile([P, DCH * SQ], bf16, name="xTq", tag="xTq")

            with tc.tile_pool(name="stageW", bufs=1) as sw:
                wa = sw.tile([P, DCH * D], bf16, name="wa")
                wb = sw.tile([P, DCH * D], bf16, name="wb")

                # gpsimd ring: wa in 128-col slices (first G chain needs
                # only slice 0: 256KB), then xTq, then (in-loop) the masks.
                gp_dmas = []
                for s8 in range(DCH):
                    lo, hi = s8 * P, (s8 + 1) * P
                    gp_dmas.append(nc.gpsimd.dma_start(
                        wa[:].rearrange("p (c d) -> p c d", c=DCH)[:, :, lo:hi],
                        wqt_d.ap()[:, lo:hi].rearrange("(c p) d -> p c d", p=P)))
                gp_dmas.append(nc.gpsimd.dma_start(
                    xTq[:].rearrange("p (c r) -> p c r", c=DCH),
                    xTq_d.ap().rearrange("(c p) r -> p c r", p=P)))

                # sync ring: wb in 256-col slices for jb=0 (first half-chain
                # gated on 512KB), 512 for jb=1; then xTs.
                sy_dmas = []
                for lo, hi in ((0, 256), (256, 512), (512, 1024)):
                    sy_dmas.append(nc.sync.dma_start(
                        wb[:].rearrange("p (c d) -> p c d", c=DCH)[:, :, lo:hi],
                        wkt_d.ap()[:, lo:hi].rearrange("(c p) d -> p c d", p=P)))
                sy_dmas.append(nc.sync.dma_start(
                    xTs[:].rearrange("p (c r) -> p c r", c=DCH),
                    xT_d.ap().rearrange("(c p) r -> p c r", p=P)))
                if use_deps:
                    for ring in (gp_dmas, sy_dmas):
                        for a, b in zip(ring[1:], ring[:-1]):
                            add_dep_helper(a.ins, b.ins, sync=False,
                                           reason="dma order")

                # G[i,j] = sum_d Wq[i,d] Wk[j,d]: lhsT=WqT slice, rhs=WkT.
                # jb=0 runs as two 256-wide half-chains into one PSUM tile
                # so the very first matmul needs only wb cols [0:256).
                for jb in range(D // NB):
                    for ic in range(DCH):
                        ps = psp.tile([P, NB], fp32, tag="ps", name="ps")
                        halves = ((0, 256), (256, 512)) if jb == 0 else ((0, NB),)
                        for hlo, hhi in halves:
                            w = hhi - hlo
                            for c in range(DCH):
                                nc.tensor.matmul(
                                    ps[:, hlo:hhi],
                                    wa[:, c * D + ic * P: c * D + (ic + 1) * P],
                                    wb[:, c * D + jb * NB + hlo:
                                       c * D + jb * NB + hhi],
                                    start=(c == 0), stop=(c == DCH - 1))
                        drain(Gs[:, ic * D + jb * NB: ic * D + (jb + 1) * NB], ps[:])

            # stageW (wa/wb) closed; the work pool reuses its bytes (the
            # early mask/Pt writes WAR-wait on the last G-chain reads).
            wk_ctx = tc.tile_pool(name="work", bufs=2)
            wkp = wk_ctx.__enter__()

            # TT[j,q] = sum_i G[i,j] xq[q,i]: lhsT=G chunk, rhs=xTq.
            # Emitted per 512-q block so scores(0-3) start after TTb0.
            def tt_block(qb):
                for jc in range(DCH):
                    ps = psp.tile([P, NB], fp32, tag="ps", name="ps")
                    for ic in range(DCH):
                        nc.tensor.matmul(
                            ps[:],
                            Gs[:, ic * D + jc * P: ic * D + (jc + 1) * P],
                            xTq[:, ic * SQ + qb * NB: ic * SQ + (qb + 1) * NB],
                            start=(ic == 0), stop=(ic == DCH - 1))
                    drain(TT[:, jc * SQ + qb * NB: jc * SQ + (qb + 1) * NB],
                          ps[:])

            def mt_load(t):
                mt = wkp.tile([P, M * S], u8, tag="mt", name=f"mt{t}",
                              bufs=3)
                nc.gpsimd.dma_start(
                    mt[:].rearrange("p (m k) -> p m k", m=M),
                    mk_d.ap()[:, t * P:(t + 1) * P, :].transpose([1, 0, 2]))
                return mt

            inv_scale = 1.0 / float(np.sqrt(np.float32(D)))
            pts = {}

            def sc_exp(t):
                """scores tile -> exp -> Pt."""
                Pt = wkp.tile([P, S], bf16, tag="Pt", name="Pt", bufs=4)
                for kb in range(S // NB):
                    ps = psp.tile([P, NB], fp32, tag="ps", name="ps")
                    for c in range(DCH):
                        nc.tensor.matmul(
                            ps[:],
                            TT[:, c * SQ + t * P: c * SQ + (t + 1) * P],
                            xTs[:, c * S + kb * NB: c * S + (kb + 1) * NB],
                            start=(c == 0), stop=(c == DCH - 1))
                    nc.scalar.activation(
                        Pt[:, kb * NB:(kb + 1) * NB], ps[:],
                        AF.Exp, scale=inv_scale)
                pts[t] = Pt

            mts = {0: mt_load(0), 1: mt_load(1)}
            tt_block(0)
            for t in range(4):
                sc_exp(t)
            tt_block(1)

            # wv/wo reuse the Gs/xTq tags (same 16KB slots): their DMAs
            # WAR-wait on TTb1's last reads, landing ~115us — before the
            # H chunks need them at ~127us.
            wv2 = sg.tile([P, DCH * D], bf16, name="wv2", tag="Gs")
            wo2 = sg.tile([P, DCH * D], bf16, name="wo2", tag="xTq")
            d_wv = nc.sync.dma_start(
                wv2[:].rearrange("p (c d) -> p c d", c=DCH),
                wvt_d.ap().rearrange("(c p) d -> p c d", p=P))
            d_wo = nc.sync.dma_start(
                wo2[:].rearrange("p (c d) -> p c d", c=DCH),
                wo_d.ap().rearrange("(c p) d -> p c d", p=P))
            if use_deps:
                add_dep_helper(d_wv.ins, sy_dmas[-1].ins, sync=False,
                               reason="dma order")
                add_dep_helper(d_wo.ins, d_wv.ins, sync=False,
                               reason="dma order")

            def c_prod(t):
                """tile t products: fused mask*P + row-sums + recip, on DVE.
                GpSimd does NO elementwise work in phase C: it shares SBUF
                ports with DVE, so concurrent gp ops halve DVE throughput."""
                if t + 2 < QTILES:
                    mts[t + 2] = mt_load(t + 2)
                mt = mts.pop(t)
                Pt = pts.pop(t)

                den = wkp.tile([P, M], fp32, tag="den", name="den")
                Tm = [wkp.tile([P, S], bf16, tag=f"Tm{m}", name=f"Tm{m}",
                               bufs=2)
                      for m in range(M)]
                for m in range(M):
                    nc.vector.scalar_tensor_tensor(
                        out=Tm[m][:],
                        in0=mt[:, m * S:(m + 1) * S],
                        scalar=1.0, in1=Pt[:],
                        op0=ALU.mult, op1=ALU.mult,
                        accum_out=den[:, m:m + 1])
                inv = wkp.tile([P, M], fp32, tag="inv", name="inv")
                nc.vector.reciprocal(inv[:], den[:])
                return inv, Tm

            def c_scale(t, inv, Tm):
                """per-mask inv scaling on ACT (own SBUF ports)."""
                for m in range(M):
                    nc.scalar.activation(Tm[m][:], Tm[m][:], AF.Copy,
                                         scale=inv[:, m:m + 1])
                return Tm

            def c_tail(t, Tm):
                nc.vector.tensor_add(Tm[0][:], Tm[0][:], Tm[1][:])
                nc.vector.tensor_add(Tm[2][:], Tm[2][:], Tm[3][:])
                nc.vector.tensor_add(Tm[0][:], Tm[0][:], Tm[2][:])
                # transpose Wsum [128, S] -> WT column t via xbar DMA
                nc.sync.dma_start_transpose(
                    WT[:].rearrange("p (i q) -> p i q", i=KCH)
                    [:, :, t * P:(t + 1) * P],
                    Tm[0][:])

            def h_chunk(k):
                # 2 ic-blocks of H = Wv Wo per call; drains on ACT where
                # they interleave with the scales (DVE is softmax-busy).
                for ic in (2 * k, 2 * k + 1):
                    for ob in range(D // NB):
                        ps = psp.tile([P, NB], fp32, tag="ps", name="ps")
                        for c in range(DCH):
                            nc.tensor.matmul(
                                ps[:],
                                wv2[:, c * D + ic * P: c * D + (ic + 1) * P],
                                wo2[:, c * D + ob * NB: c * D + (ob + 1) * NB],
                                start=(c == 0), stop=(c == DCH - 1))
                        drain(Hs[:, ic * D + ob * NB: ic * D + (ob + 1) * NB],
                              ps[:], eng=nc.scalar)

            # software pipeline: products(t) | tail(t-1) | scale(t) | exp(t+4)
            xks = None
            prev = None
            for t in range(QTILES):
                inv, Tm = c_prod(t)
                if prev is not None:
                    c_tail(t - 1, prev)
                prev = c_scale(t, inv, Tm)
                if t + 4 < QTILES:
                    sc_exp(t + 4)
                if t == 5:
                    # xk refill of the xmem slot on the gp ring, behind
                    # mt(7): fires once the last scores matmul frees xTs
                    # (~130us), lands before f0 (~155us).
                    xks = pp.tile([P, KCH * D], bf16, tag="xmem", name="xks")
                    nc.gpsimd.dma_start(
                        xks[:].rearrange("p (i d) -> p i d", i=KCH),
                        xk_d.ap().rearrange("(i p) d -> p i d", p=P))
                if t >= 4:
                    h_chunk(t - 4)
            c_tail(QTILES - 1, prev)
            wk_ctx.__exit__(None, None, None)

            # ---- phases F/G ---------------------------------------------
            fg_ctx = tc.tile_pool(name="fg", bufs=1)
            fgp = fg_ctx.__enter__()
            OT = fgp.tile([P, DCH * NB], bf16, name="OT")  # [p,c*NB+qc]=U[qb*NB+qc,c*128+p]

            def f_block(qb, eng=None):
                # OT[j, qc] = sum_k x[k, j*128+jj] Wsum[qb*NB+qc, k]
                for j in range(DCH):
                    ps = psp.tile([P, NB], fp32, tag="ps", name="ps")
                    for i in range(KCH):
                        nc.tensor.matmul(
                            ps[:],
                            xks[:, i * D + j * P: i * D + (j + 1) * P],
                            WT[:, i * SQ + qb * NB: i * SQ + (qb + 1) * NB],
                            start=(i == 0), stop=(i == KCH - 1))
                    drain(OT[:, j * NB:(j + 1) * NB], ps[:], eng=eng)

            def g_tile(t):
                ot = fgp.tile([P, D], fp32, tag="ot", name="ot", bufs=2)
                for ob in range(D // NB):
                    ps = psp.tile([P, NB], fp32, tag="ps", name="ps")
                    for c in range(DCH):
                        nc.tensor.matmul(
                            ps[:],
                            OT[:, c * NB + (t % 4) * P: c * NB + (t % 4 + 1) * P],
                            Hs[:, c * D + ob * NB: c * D + (ob + 1) * NB],
                            start=(c == 0), stop=(c == DCH - 1))
                    drain(ot[:, ob * NB:(ob + 1) * NB], ps[:])
                nc.sync.dma_start(out_d.ap()[t * P:(t + 1) * P, :], ot[:])

            # f0's drains all on ACT: its scale stream has idle gaps while
            # DVE (the softmax pacer) is still busy when f0's PSUMs land.
            f_block(0, eng=nc.scalar)
            for t in range(4):
                g_tile(t)
            f_block(1)
            for t in range(4, 8):
                g_tile(t)
            fg_ctx.__exit__(None, None, None)
            stageg_ctx.__exit__(None, None, None)
            wt_ctx.__exit__(None, None, None)

    nc.compile()
    return nc


def _get_nc():
    if "nc" not in _CACHE:
        _CACHE["nc"] = build()
    return _CACHE["nc"]


def kernel(x, stride_masks, Wq, bq, Wk, bk, Wv, bv, Wo, bo):
    import ml_dtypes
    from concourse import bass_utils

    bf16 = ml_dtypes.bfloat16
    x = np.ascontiguousarray(np.asarray(x, dtype=np.float32))
    stride_masks = np.asarray(stride_masks, dtype=np.int32)
    Wq = np.asarray(Wq, dtype=np.float32)
    Wk = np.asarray(Wk, dtype=np.float32)
    Wv = np.asarray(Wv, dtype=np.float32)
    Wo = np.asarray(Wo, dtype=np.float32)
    bq = np.asarray(bq, dtype=np.float32)
    bk = np.asarray(bk, dtype=np.float32)
    bv = np.asarray(bv, dtype=np.float32)
    bo = np.asarray(bo, dtype=np.float32)

    nc = _get_nc()

    # Biases are spec'd zero-fill; the device kernel omits them. bv/bo fold
    # in exactly on the host (softmax rows sum to 1); bq/bk would need a
    # device path, so assert they are zero.
    assert not (np.any(bq) or np.any(bk)), "nonzero q/k bias unsupported"

    mk_u8 = stride_masks.astype(np.uint8)
    mk_half = [np.ascontiguousarray(mk_u8[:, h * SQ:(h + 1) * SQ, :])
               for h in range(2)]
    wqt = Wq.T.astype(bf16)
    wkt = Wk.T.astype(bf16)
    wvt = (Wv.T / np.float32(M)).astype(bf16)   # folds the mask-mean 1/M
    wo16 = Wo.astype(bf16)
    xT_bf = [x[b].T.astype(bf16) for b in range(B)]
    xk_bf = [x[b].astype(bf16) for b in range(B)]

    in_maps = []
    for c in range(N_CORES):
        b, h = c // 2, c % 2
        in_maps.append({
            "xT": xT_bf[b],
            "xTq": np.ascontiguousarray(xT_bf[b][:, h * SQ:(h + 1) * SQ]),
            "xk": xk_bf[b], "mk": mk_half[h],
            "wqt": wqt, "wkt": wkt, "wvt": wvt, "wo": wo16,
        })

    res = bass_utils.run_bass_kernel_spmd(nc, in_maps, core_ids=list(range(N_CORES)))
    _CACHE["last_results"] = res

    out = np.empty((B, S, D), dtype=np.float32)
    for c in range(N_CORES):
        b, h = c // 2, c % 2
        out[b, h * SQ:(h + 1) * SQ, :] = res.results[c]["out"]

    if np.any(bv):
        out += (bv @ Wo)[None, None, :]
    if np.any(bo):
        out += bo[None, None, :]
    return out
